# revision 5
# baseline (speedup 1.0000x reference)
"""NT-Xent / contrastive loss on 8 Trainium2 NeuronCores.

Reference computation (B=4096, D=512, temp=0.1):
    z   = l2norm(concat(proj_1, proj_2))          # [8192, 512]
    cos = (z @ z.T) / temp                        # [8192, 8192]
    pos[r]  = cos[r, (r + 4096) % 8192]
    lse[r]  = logsumexp(cos[r, :] with cos[r, r] masked out)
    loss    = mean(lse - pos)

Sharding: rows of the similarity matrix, 1024 per core.  The host
normalizes z, scales by S=64 and quantizes to fp8-e4m3, then ships each
core the full z^T *rotated* by core*1024 rows in GEMM-ready layout
[128, 4*8192] (K-chunk k at columns [k*8192, (k+1)*8192)).  The rotation
makes the program uniform across cores (SPMD): local rows 0..1023 are
the core's rows, the self-diagonal sits at local column == row, and the
positive sits at local column == row + 4096.

Per core:
  1. Stream the fp8 z^T in 16 DMA chunks (column-group-major so the
     GEMM can start after the first 4).
  2. GEMM: for each 128-row block m and 2048-col group J, accumulate
     8 fp8 DoubleRow matmuls (4 column chunks x 2 k-pairs, 2 K-tiles
     per instruction; measured ~2x the bf16 stream rate) into a 4-bank
     PSUM tile, then a single ScalarE Exp(scale=10/S^2) downcasts to a
     bf16 SBUF tile.  ScalarE is the critical engine (1 elem/cycle/lane
     regardless of dtype), so everything else is kept off it: row sums
     run on DVE (2x bf16 mode) and the self/positive diagonals are
     pulled out of the *exp'd* bf16 tile with a multiply-by-identity
     reduce (J==0 holds exp(A*self), J==2 holds exp(A*pos), both at
     column offset m*128, thanks to the input rotation).  Post-exp
     extraction keeps ScalarE free of the WAR interlock a raw-PSUM
     read would impose.
  3. loss_r = ln(sumexp_r - exps_r) - ln(expp_r): the exp'd self value
     is subtracted exactly (same bf16 value that entered the row sum),
     and ln(exp(A*pos)) recovers A*pos.  Two Ln's with accum_out give
     sum(lse) and sum(A*pos) per partition; a ones-matmul reduces the
     difference to the [1,1] partial.
Host adds the 8 partials and divides by 8192.

fp8 error budget: z elements ~N(0, 1/512); e4m3 keeps ~2-3% relative
per element, the 1024-term dot product error is ~2e-3 rms, i.e. ~0.02
on the exponent after the 1/temp scale; the loss averages 8192 rows so
the net relative error is ~1e-4, far under the 2e-2 gate.  The exp of
the self-similarity is cancelled exactly: the same PSUM fp32 value goes
through the same ScalarE Exp in phase 2 (summed) and phase 3
(subtracted), so quantization does not perturb the masking.
"""

import sys

import ml_dtypes
import numpy as np

if "/opt/trn_rl_repo" not in sys.path:
    sys.path.insert(0, "/opt/trn_rl_repo")

_B = 4096
_D = 512
_N2 = 2 * _B            # 8192 rows of the similarity matrix
_NCORES = 8
_RPC = _N2 // _NCORES   # 1024 rows per core
_INV_TEMP = 10.0
_S = 64.0               # fp8 quantization scale for normalized z
_A = _INV_TEMP / (_S * _S)   # exp() scale on raw fp8 GEMM accumulators

_NM = _RPC // 128       # 8 output row blocks per core
_NK = _D // 128         # 4 contraction chunks
_NJG = 4                # 4 column groups of 2048
_JG = _N2 // _NJG       # 2048 columns per group


def _emit(tc, ztq, out_partial):
    import concourse.bass as bass  # noqa: F401
    from concourse import mybir

    nc = tc.nc
    f32 = mybir.dt.float32
    Alu = mybir.AluOpType
    Act = mybir.ActivationFunctionType

    from contextlib import ExitStack
    ctx = ExitStack()
    pool = ctx.enter_context(tc.tile_pool(name="work", bufs=1))
    pers = ctx.enter_context(tc.tile_pool(name="pers", bufs=1))
    pspool = ctx.enter_context(tc.tile_pool(name="psum", bufs=1, space="PSUM"))

    # ---- constants ----
    ones = pers.tile([128, 128], f32, tag="ones")
    nc.vector.memset(ones[:], 1.0)
    ident = pers.tile([128, 128], f32, tag="ident")
    nc.gpsimd.affine_select(ident[:], ones[:], pattern=[[1, 128]],
                            compare_op=Alu.is_equal, fill=0.0,
                            base=0, channel_multiplier=-1)
    identb = pers.tile([128, 128], mybir.dt.bfloat16, tag="identb")
    nc.vector.tensor_copy(identb[:], ident[:])

    # ---- persistent buffers ----
    zsb = pers.tile([128, _NK * _N2], mybir.dt.float8e4, tag="zt")
    zt3 = zsb.rearrange("p (k c) -> p k c", k=_NK)
    sp_all = pers.tile([128, 2 * _NM], f32, tag="sp")    # self diag | pos diag
    rs_all = pers.tile([128, _NM], f32, tag="rs")        # row sumexp per block

    # ---- input DMA: column-group-major so group 0 lands first ----
    for jg in range(_NJG):
        for k in range(_NK):
            nc.sync.dma_start(
                zt3[:, k, jg * _JG:(jg + 1) * _JG],
                ztq[:, k * _N2 + jg * _JG: k * _N2 + (jg + 1) * _JG])

    # ---- main loop: fp8 DoubleRow GEMM + exp + row sums ----
    for m in range(_NM):
        se = pool.tile([128, _NJG], f32, tag="se", bufs=2, name=f"se{m}")
        off = m * 128
        for J in range(_NJG):
            ps = pspool.tile([128, _JG], f32, tag="ps", bufs=2,
                             name=f"ps{m}_{J}")
            for kp in range(2):
                for c in range(4):
                    nc.tensor.matmul(
                        ps[:, c * 512:(c + 1) * 512],
                        zt3[:, 2 * kp:2 * kp + 2, off:off + 128],
                        zt3[:, 2 * kp:2 * kp + 2,
                            J * _JG + c * 512:J * _JG + (c + 1) * 512],
                        start=(kp == 0), stop=(kp == 1),
                        perf_mode=mybir.MatmulPerfMode.DoubleRow)
            eb = pool.tile([128, _JG], mybir.dt.bfloat16, tag="eb", bufs=3,
                           name=f"eb{m}_{J}")
            nc.scalar.activation(eb[:], ps[:], Act.Exp, bias=0.0, scale=_A)
            nc.vector.reduce_sum(out=se[:, J:J + 1], in_=eb[:],
                                 axis=mybir.AxisListType.X)
            if J == 0 or J == 2:
                col = m if J == 0 else _NM + m
                junk = pool.tile([128, 128], mybir.dt.bfloat16, tag="junk",
                                 bufs=2, name=f"junk{m}_{J}")
                nc.vector.scalar_tensor_tensor(
                    out=junk[:], in0=eb[:, off:off + 128], scalar=1.0,
                    in1=identb[:], op0=Alu.mult, op1=Alu.mult,
                    accum_out=sp_all[:, col:col + 1])
        nc.vector.reduce_sum(out=rs_all[:, m:m + 1], in_=se[:],
                             axis=mybir.AxisListType.X)

    # ---- phase 3: loss_r = ln(sumexp_r - exps_r) - ln(expp_r) ----
    nc.vector.tensor_sub(rs_all[:], rs_all[:], sp_all[:, 0:_NM])
    la = pool.tile([128, 1], f32, tag="la")
    lse = pool.tile([128, _NM], f32, tag="lse")
    nc.scalar.activation(lse[:], rs_all[:], Act.Ln, bias=0.0, scale=1.0,
                         accum_out=la[:])
    lp = pool.tile([128, 1], f32, tag="lp")
    lpx = pool.tile([128, _NM], f32, tag="lpx")
    nc.scalar.activation(lpx[:], sp_all[:, _NM:2 * _NM], Act.Ln, bias=0.0,
                         scale=1.0, accum_out=lp[:])
    lossv = pool.tile([128, 1], f32, tag="lossv")
    nc.vector.tensor_sub(lossv[:], la[:], lp[:])
    pf = pspool.tile([1, 1], f32, tag="ps", bufs=2)
    nc.tensor.matmul(pf[:], lossv[:], ones[:, 0:1], start=True, stop=True)
    res = pool.tile([1, 1], f32, tag="res")
    nc.vector.tensor_copy(res[:], pf[:])
    nc.sync.dma_start(out_partial[:, :], res[:])

    ctx.close()


def build():
    import concourse.tile as tile
    from concourse import bacc, mybir

    nc = bacc.Bacc("TRN2", target_bir_lowering=False, debug=False,
                   enable_asserts=True, num_devices=_NCORES)
    ztq = nc.dram_tensor("ztq", [128, _NK * _N2], mybir.dt.float8e4,
                         kind="ExternalInput").ap()
    out_partial = nc.dram_tensor("partial", [1, 1], mybir.dt.float32,
                                 kind="ExternalOutput").ap()
    with tile.TileContext(nc) as tc:
        _emit(tc, ztq, out_partial)
    nc.compile()
    return nc


_NC_CACHE = None


def _get_nc():
    global _NC_CACHE
    if _NC_CACHE is None:
        _NC_CACHE = build()
    return _NC_CACHE


def make_in_maps(proj_1, proj_2):
    z = np.concatenate([np.asarray(proj_1, dtype=np.float32),
                        np.asarray(proj_2, dtype=np.float32)], axis=0)
    n = np.sqrt((z * z).sum(axis=1, keepdims=True))
    zq = ((z / np.maximum(n, 1e-12)) * _S).astype(ml_dtypes.float8_e4m3)
    in_maps = []
    for c in range(_NCORES):
        zr = np.roll(zq, -_RPC * c, axis=0)          # [8192, 512]
        zt = zr.T.reshape(_NK, 128, _N2)             # [k, p, col]
        ztq = np.ascontiguousarray(
            zt.transpose(1, 0, 2).reshape(128, _NK * _N2))
        in_maps.append({"ztq": ztq})
    return in_maps


def kernel(proj_1, proj_2):
    from concourse import bass_utils

    nc = _get_nc()
    in_maps = make_in_maps(proj_1, proj_2)
    r = bass_utils.run_bass_kernel_spmd(nc, in_maps,
                                        core_ids=list(range(_NCORES)))
    total = sum(float(res["partial"][0, 0]) for res in r.results)
    return np.float32(total / _N2)


# revision 7
# speedup vs baseline: 1.4736x; 1.4736x over previous
"""NT-Xent / contrastive loss on 8 Trainium2 NeuronCores.

Reference computation (B=4096, D=512, temp=0.1):
    z   = l2norm(concat(proj_1, proj_2))          # [8192, 512]
    cos = (z @ z.T) / temp                        # [8192, 8192]
    pos[r]  = cos[r, (r + 4096) % 8192]
    lse[r]  = logsumexp(cos[r, :] with cos[r, r] masked out)
    loss    = mean(lse - pos)

cos is symmetric, so each unordered pair of 512-row blocks is computed
ONCE: with the 16x16 grid of 512x512 blocks, core c covers local block
rows {0, 1} x local column blocks {br..br+8} on input *rotated* by
c*1024 rows.  The union over cores covers every unordered block pair
{B, B+j} exactly once for j=1..8 (j=8 pairs are computed from both
orientations, each contributing only its own rows' sums), and every
diagonal block once.  Row sums for the computed half come from ScalarE
accum_out; the transposed half's contributions are column sums via
fp8-e5m2 ones-matmuls (DoubleRow, two 128-row planes per instruction).
The host gathers per-core partial vectors, un-rotates, adds the row- and
column-sum halves, and finishes the (tiny) lse/loss reduction in fp64.

Device-side layout per core (all in local rotated coordinates):
  - input: z^T normalized, scaled by S=64, fp8-e4m3, GEMM-ready
    [128, 4*5120] (K-chunk k at columns [k*5120,(k+1)*5120)) -- only
    columns 0..5119 are touched (block rows 0,1 x their 9 chunks).
    Three 3D-AP DMAs (all 4 K-chunks per trigger) keep the Sync engine
    off the critical path.
  - for block-row br in {0,1}, row-subtile a in 0..3: 3 column groups
    of 1536 (9 chunks of 512: local blocks br..br+8): 6 fp8-e4m3
    DoubleRow matmuls -> [128,1536] PSUM; self diag (group 0, offset
    a*128) and positive diag (group 2, offset 1024+a*128) are pulled
    from raw PSUM with multiply-by-identity reduces (pure reads, no
    WAR: exp writes to SBUF); ScalarE Exp(scale=10/S^2) writes e5m2 to
    SBUF with accum_out row sums.
  - column sums: for chunks j=1..7 (j=0 self-block and j=8 positive
    chunk are row-sum-only), a ones-DoubleRow matmul over the two
    row-subtile planes -> [128,512] PSUM (all partitions equal), DVE
    evacuates partition 0 into a packed [1, 28*512] vector.  Each
    group's column sums are emitted two GEMM steps late so PE never
    waits on ScalarE and the DVE evacuations spread out.
Outputs: rsp [128,40] (24 row sums | 8 raw self diag | 8 raw pos
diag), cs [1,14336] (column sums).  ~80KB total DMA out.

fp8-e4m3 GEMM error keeps the loss within ~1e-4 of fp64; e5m2 on the
exp'd values only feeds column sums (relative rms error ~6%/sqrt(4096)
per row total).  The self term is subtracted on the host as
exp(A*self_raw) with A=10/S^2, matching the ScalarE exp of the same
fp32 PSUM value to table accuracy, harmless at the 2e-2 gate.
"""

import sys

import ml_dtypes
import numpy as np

if "/opt/trn_rl_repo" not in sys.path:
    sys.path.insert(0, "/opt/trn_rl_repo")

_B = 4096
_D = 512
_N2 = 2 * _B            # 8192 rows of the similarity matrix
_NCORES = 8
_RPC = _N2 // _NCORES   # 1024 rows per core
_INV_TEMP = 10.0
_S = 64.0               # fp8 quantization scale for normalized z
_A = _INV_TEMP / (_S * _S)   # exp() scale on raw fp8 GEMM accumulators

_NK = _D // 128         # 4 contraction chunks
_NBR = 2                # 512-row block-rows per core
_NA = 4                 # 128-row subtiles per block-row
_NG = 3                 # column groups per (br, a)
_GW = 1536              # columns per group (3 chunks of 512)
_W = 5120               # zT columns resident per core (blocks 0..9)

# column-sum chunks per group (skip j=0 self block and j=8 positive)
_CS_CHUNKS = {0: (1, 2), 1: (3, 4, 5), 2: (6, 7)}
_CS_POS = {}            # (g, j) -> slot within a (br, apair) 7-vector
for _g in range(_NG):
    for _j in _CS_CHUNKS[_g]:
        _CS_POS[(_g, _j)] = len(_CS_POS)
_NCS = 4 * 7            # (br, apair) combos x 7 chunks


def _emit(tc, ztq, rsp_out, cs_out):
    import concourse.bass as bass  # noqa: F401
    from concourse import mybir

    nc = tc.nc
    f32 = mybir.dt.float32
    e5 = mybir.dt.float8e5
    Alu = mybir.AluOpType
    Act = mybir.ActivationFunctionType
    DR = mybir.MatmulPerfMode.DoubleRow

    from contextlib import ExitStack
    ctx = ExitStack()
    pool = ctx.enter_context(tc.tile_pool(name="work", bufs=1))
    pers = ctx.enter_context(tc.tile_pool(name="pers", bufs=1))
    pspool = ctx.enter_context(tc.tile_pool(name="psum", bufs=1, space="PSUM"))

    # ---- constants ----
    ones = pers.tile([128, 128], f32, tag="ones")
    nc.vector.memset(ones[:], 1.0)
    ident = pers.tile([128, 128], f32, tag="ident")
    nc.gpsimd.affine_select(ident[:], ones[:], pattern=[[1, 128]],
                            compare_op=Alu.is_equal, fill=0.0,
                            base=0, channel_multiplier=-1)
    onese5 = pers.tile([128, 2, 128], e5, tag="onese5")
    nc.vector.memset(onese5[:], 1.0)

    # ---- persistent buffers ----
    zsb = pers.tile([128, _NK * _W], mybir.dt.float8e4, tag="zt")
    zt3 = zsb.rearrange("p (k c) -> p k c", k=_NK)
    rsp = pers.tile([128, 40], f32, tag="rsp")  # 24 rowsums | 8 self | 8 pos
    cs_sb = pers.tile([1, _NCS * 512], f32, tag="cs")

    # ---- input DMA: one 3D transfer per column range ----
    ztq3 = ztq.rearrange("p (k c) -> p k c", k=_NK)
    for c0, c1 in ((0, 2048), (2048, 4096), (4096, _W)):
        nc.sync.dma_start(zt3[:, :, c0:c1], ztq3[:, :, c0:c1])

    # ---- main loop ----
    def emit_colsums(br, apair, g, ebt):
        slot0 = (br * 2 + apair) * 7
        for j in _CS_CHUNKS[g]:
            cchunk = j - 3 * g
            idx = slot0 + _CS_POS[(g, j)]
            cst = pspool.tile([128, 512], f32, tag="cs", bufs=2,
                              name=f"cs{br}_{apair}_{j}")
            nc.tensor.matmul(
                cst[:], onese5[:],
                ebt[:, :, cchunk * 512:(cchunk + 1) * 512],
                start=True, stop=True, perf_mode=DR)
            nc.vector.tensor_copy(
                cs_sb[0:1, idx * 512:(idx + 1) * 512], cst[0:1, :])

    pending = []        # (ready_step, br, apair, g, ebp tile)
    step = 0
    for br in range(_NBR):
        base = br * 512
        for apair in range(2):
            ebp = []
            for g in range(_NG):
                ebp.append(pool.tile([128, 2, _GW], e5, tag=f"eb{g}",
                                     bufs=2, name=f"eb{br}_{apair}_{g}"))
            for g in range(_NG):
                for ai in range(2):
                    a = apair * 2 + ai
                    lo = base + a * 128
                    ps = pspool.tile([128, _GW], f32, tag="ps", bufs=2,
                                     name=f"ps{br}_{a}_{g}")
                    for cc in range(3):
                        col = base + (g * 3 + cc) * 512
                        for kp in range(2):
                            nc.tensor.matmul(
                                ps[:, cc * 512:(cc + 1) * 512],
                                zt3[:, 2 * kp:2 * kp + 2, lo:lo + 128],
                                zt3[:, 2 * kp:2 * kp + 2, col:col + 512],
                                start=(kp == 0), stop=(kp == 1),
                                perf_mode=DR)
                    if g == 0 or g == 2:
                        off = a * 128 if g == 0 else 1024 + a * 128
                        col_s = 24 + (br * _NA + a) + (0 if g == 0 else 8)
                        junk = pool.tile([128, 128], f32, tag="junk",
                                         bufs=2, name=f"junk{br}_{a}_{g}")
                        nc.vector.scalar_tensor_tensor(
                            out=junk[:], in0=ps[:, off:off + 128], scalar=1.0,
                            in1=ident[:], op0=Alu.mult, op1=Alu.mult,
                            accum_out=rsp[:, col_s:col_s + 1])
                    slot = (br * _NA + a) * _NG + g
                    nc.scalar.activation(ebp[g][:, ai, :], ps[:], Act.Exp,
                                         bias=0.0, scale=_A,
                                         accum_out=rsp[:, slot:slot + 1])
                    step += 1
                    while pending and pending[0][0] <= step:
                        _, pbr, pap, pg, pebt = pending.pop(0)
                        emit_colsums(pbr, pap, pg, pebt)
                pending.append((step + 2, br, apair, g, ebp[g]))
    while pending:
        _, pbr, pap, pg, pebt = pending.pop(0)
        emit_colsums(pbr, pap, pg, pebt)

    # ---- output DMA ----
    nc.sync.dma_start(rsp_out[:, :], rsp[:])
    nc.sync.dma_start(cs_out[:, :], cs_sb[:])

    ctx.close()


def build():
    import concourse.tile as tile
    from concourse import bacc, mybir

    nc = bacc.Bacc("TRN2", target_bir_lowering=False, debug=False,
                   enable_asserts=True, num_devices=_NCORES)
    ztq = nc.dram_tensor("ztq", [128, _NK * _W], mybir.dt.float8e4,
                         kind="ExternalInput").ap()
    rsp_out = nc.dram_tensor("rsp", [128, 40], mybir.dt.float32,
                             kind="ExternalOutput").ap()
    cs_out = nc.dram_tensor("cs", [1, _NCS * 512], mybir.dt.float32,
                            kind="ExternalOutput").ap()
    with tile.TileContext(nc) as tc:
        _emit(tc, ztq, rsp_out, cs_out)
    nc.compile()
    return nc


_NC_CACHE = None


def _get_nc():
    global _NC_CACHE
    if _NC_CACHE is None:
        _NC_CACHE = build()
    return _NC_CACHE


def make_in_maps(proj_1, proj_2):
    z = np.concatenate([np.asarray(proj_1, dtype=np.float32),
                        np.asarray(proj_2, dtype=np.float32)], axis=0)
    n = np.sqrt((z * z).sum(axis=1, keepdims=True))
    zq = ((z / np.maximum(n, 1e-12)) * _S).astype(ml_dtypes.float8_e4m3)
    in_maps = []
    for c in range(_NCORES):
        zr = np.roll(zq, -_RPC * c, axis=0)          # [8192, 512]
        zt = zr.T.reshape(_NK, 128, _N2)             # [k, p, col]
        ztq = np.ascontiguousarray(
            zt[:, :, 0:_W].transpose(1, 0, 2).reshape(128, _NK * _W))
        in_maps.append({"ztq": ztq})
    return in_maps


def _combine(results):
    """Host-side gather: un-rotate per-core partial sums, finish lse."""
    totals = np.zeros(_N2, dtype=np.float64)
    self_raw = np.zeros(_N2, dtype=np.float64)
    pos_raw = np.zeros(_N2, dtype=np.float64)
    p_idx = np.arange(128)
    q_idx = np.arange(512)
    for c, res in enumerate(results):
        rot = _RPC * c
        rsp = np.asarray(res["rsp"], dtype=np.float64)    # [128, 40]
        cs = np.asarray(res["cs"], dtype=np.float64)[0]   # [14336]
        for br in range(_NBR):
            for apair in range(2):
                slot0 = (br * 2 + apair) * 7
                for g in range(_NG):
                    for j in _CS_CHUNKS[g]:
                        idx = slot0 + _CS_POS[(g, j)]
                        b = br + j  # local column block
                        gr = (512 * b + q_idx + rot) % _N2
                        totals[gr] += cs[idx * 512:(idx + 1) * 512]
        for br in range(_NBR):
            for a in range(_NA):
                gr = (512 * br + 128 * a + p_idx + rot) % _N2
                s = br * _NA + a
                totals[gr] += rsp[:, s * _NG:(s + 1) * _NG].sum(axis=1)
                self_raw[gr] = rsp[:, 24 + s]
                pos_raw[gr] = rsp[:, 32 + s]
    lse = np.log(totals - np.exp(_A * self_raw))
    loss = np.mean(lse - _A * pos_raw)
    return np.float32(loss)


def kernel(proj_1, proj_2):
    from concourse import bass_utils

    nc = _get_nc()
    in_maps = make_in_maps(proj_1, proj_2)
    r = bass_utils.run_bass_kernel_spmd(nc, in_maps,
                                        core_ids=list(range(_NCORES)))
    return _combine(r.results)


# revision 15
# speedup vs baseline: 1.5369x; 1.0430x over previous
"""NT-Xent / contrastive loss on 8 Trainium2 NeuronCores.

Reference computation (B=4096, D=512, temp=0.1):
    z   = l2norm(concat(proj_1, proj_2))          # [8192, 512]
    cos = (z @ z.T) / temp                        # [8192, 8192]
    pos[r]  = cos[r, (r + 4096) % 8192]
    lse[r]  = logsumexp(cos[r, :] with cos[r, r] masked out)
    loss    = mean(lse - pos)

cos is symmetric, so each unordered pair of 512-row blocks is computed
ONCE: with the 16x16 grid of 512x512 blocks, core c covers local block
rows {0, 1} x local column blocks {br..br+8} on input *rotated* by
c*1024 rows.  The union over cores covers every unordered block pair
{B, B+j} exactly once for j=1..8 (j=8 pairs are computed from both
orientations, each contributing only its own rows' sums), and every
diagonal block once.  Row sums for the computed half come from ScalarE
accum_out; the transposed half's contributions are column sums via
fp8-e5m2 ones-matmuls (DoubleRow, two 128-row planes per instruction).
The host gathers per-core partial vectors, un-rotates, adds the row- and
column-sum halves, and finishes the (tiny) lse/loss reduction in fp64.

Device-side layout per core (all in local rotated coordinates):
  - input: z^T normalized, scaled by S=64, fp8-e4m3, GEMM-ready
    [128, 4*5120] (K-chunk k at columns [k*5120,(k+1)*5120)) -- only
    columns 0..5119 are touched (block rows 0,1 x their 9 chunks).
    Three 3D-AP DMAs (all 4 K-chunks per trigger) keep the Sync engine
    off the critical path.
  - for block-row br in {0,1}, row-subtile a in 0..3: 3 column groups
    of 1536 (9 chunks of 512: local blocks br..br+8): 6 fp8-e4m3
    DoubleRow matmuls -> [128,1536] PSUM; self diag (group 0, offset
    a*128) and positive diag (group 2, offset 1024+a*128) are pulled
    from raw PSUM with multiply-by-identity reduces (pure reads, no
    WAR: exp writes to SBUF); ScalarE Exp(scale=10/S^2) writes e5m2 to
    SBUF with accum_out row sums.
  - column sums: for chunks j=1..7 (j=0 self-block and j=8 positive
    chunk are row-sum-only), a ones-DoubleRow matmul over the two
    row-subtile planes -> [128,512] PSUM (all partitions equal), DVE
    evacuates partition 0 into a packed [1, 28*512] vector.  Each
    group's column sums are emitted two GEMM steps late so PE never
    waits on ScalarE and the DVE evacuations spread out.
Outputs: rsp [128,40] (24 row sums | 8 raw self diag | 8 raw pos
diag), cs [1,14336] (column sums).  ~80KB total DMA out.

fp8-e4m3 GEMM error keeps the loss within ~1e-4 of fp64; e5m2 on the
exp'd values only feeds column sums (relative rms error ~6%/sqrt(4096)
per row total).  The self term is subtracted on the host as
exp(A*self_raw) with A=10/S^2, matching the ScalarE exp of the same
fp32 PSUM value to table accuracy, harmless at the 2e-2 gate.
"""

import sys

import ml_dtypes
import numpy as np

if "/opt/trn_rl_repo" not in sys.path:
    sys.path.insert(0, "/opt/trn_rl_repo")

_B = 4096
_D = 512
_N2 = 2 * _B            # 8192 rows of the similarity matrix
_NCORES = 8
_RPC = _N2 // _NCORES   # 1024 rows per core
_INV_TEMP = 10.0
_S = 64.0               # fp8 quantization scale for normalized z
_A = _INV_TEMP / (_S * _S)   # exp() scale on raw fp8 GEMM accumulators

_NK = _D // 128         # 4 contraction chunks
_NBR = 2                # 512-row block-rows per core
_NA = 4                 # 128-row subtiles per block-row
_NG = 3                 # column groups per (br, a)
_GW = 1536              # columns per group (3 chunks of 512)
_W = 5120               # zT columns resident per core (blocks 0..9)

# column-sum chunks per group (skip j=0 self block and j=8 positive)
_CS_CHUNKS = {0: (1, 2), 1: (3, 4, 5), 2: (6, 7)}
_CS_POS = {}            # (g, j) -> slot within a (br, apair) 7-vector
for _g in range(_NG):
    for _j in _CS_CHUNKS[_g]:
        _CS_POS[(_g, _j)] = len(_CS_POS)
_NCS = 4 * 7            # (br, apair) combos x 7 chunks


def _emit(tc, ztq, rsp_out, cs_out):
    import concourse.bass as bass  # noqa: F401
    from concourse import mybir

    nc = tc.nc
    f32 = mybir.dt.float32
    e5 = mybir.dt.float8e5
    Alu = mybir.AluOpType
    Act = mybir.ActivationFunctionType
    DR = mybir.MatmulPerfMode.DoubleRow

    from contextlib import ExitStack
    ctx = ExitStack()
    pool = ctx.enter_context(tc.tile_pool(name="work", bufs=1))
    pers = ctx.enter_context(tc.tile_pool(name="pers", bufs=1))
    pspool = ctx.enter_context(tc.tile_pool(name="psum", bufs=1, space="PSUM"))

    # ---- constants ----
    ones = pers.tile([128, 128], f32, tag="ones")
    nc.vector.memset(ones[:], 1.0)
    ident = pers.tile([128, 128], f32, tag="ident")
    nc.gpsimd.affine_select(ident[:], ones[:], pattern=[[1, 128]],
                            compare_op=Alu.is_equal, fill=0.0,
                            base=0, channel_multiplier=-1)
    onese5 = pers.tile([128, 2, 128], e5, tag="onese5")
    nc.vector.memset(onese5[:], 1.0)

    # ---- persistent buffers ----
    zsb = pers.tile([128, _NK * _W], mybir.dt.float8e4, tag="zt")
    zt3 = zsb.rearrange("p (k c) -> p k c", k=_NK)
    rsp = pers.tile([128, 40], f32, tag="rsp")  # 24 rowsums | 8 self | 8 pos
    cs_sb = pers.tile([1, _NCS * 512], f32, tag="cs")

    # ---- input DMA: column ranges interleaved across the Sync and
    # GpSimd DGE queues (two hardware queues run in parallel); fine
    # 512-col granularity up front so the first GEMM group's operands
    # land as early as the queues allow, coarser ranges stream behind
    # compute ----
    ztq3 = ztq.rearrange("p (k c) -> p k c", k=_NK)
    ranges = [(0, 512), (512, 1024), (1024, 1536), (1536, 2560),
              (2560, 3584), (3584, 4608), (4608, _W)]
    for i, (c0, c1) in enumerate(ranges):
        eng = nc.sync if i % 2 == 0 else nc.gpsimd
        eng.dma_start(zt3[:, :, c0:c1], ztq3[:, :, c0:c1])

    # ---- PE clock warm-up: throwaway DoubleRow matmuls on constants
    # while the first DMA is in flight (PE ramps 0.65 -> 2.4 GHz) ----
    onesw = pers.tile([128, 2, 512], e5, tag="onesw")
    nc.vector.memset(onesw[:], 1.0)
    for i in range(10):
        warm = pspool.tile([128, 512], f32, tag="cs", bufs=2,
                           name=f"warm{i}")
        nc.tensor.matmul(warm[:], onese5[:], onesw[:],
                         start=True, stop=True, perf_mode=DR)

    # ---- main loop ----
    def emit_colsums(br, apair, g, ebt):
        slot0 = (br * 2 + apair) * 7
        for j in _CS_CHUNKS[g]:
            cchunk = j - 3 * g
            idx = slot0 + _CS_POS[(g, j)]
            cst = pspool.tile([128, 512], f32, tag="cs", bufs=2,
                              name=f"cs{br}_{apair}_{j}")
            nc.tensor.matmul(
                cst[:], onese5[:],
                ebt[:, :, cchunk * 512:(cchunk + 1) * 512],
                start=True, stop=True, perf_mode=DR)
            nc.vector.tensor_copy(
                cs_sb[0:1, idx * 512:(idx + 1) * 512], cst[0:1, :])

    pending = []        # (ready_step, br, apair, g, ebp tile)
    step = 0
    for br in range(_NBR):
        base = br * 512
        for apair in range(2):
            if br == 1 and apair == 1:
                # br0 column sums are all evacuated by now; drain them
                # early so the end-of-kernel DMA tail only carries br1
                nc.sync.dma_start(cs_out[:, 0:14 * 512],
                                  cs_sb[0:1, 0:14 * 512])
            ebp = []
            for g in range(_NG):
                ebp.append(pool.tile([128, 2, _GW], e5, tag=f"eb{g}",
                                     bufs=2, name=f"eb{br}_{apair}_{g}"))
            for g in range(_NG):
                for ai in range(2):
                    a = apair * 2 + ai
                    lo = base + a * 128
                    ps = pspool.tile([128, _GW], f32, tag="ps", bufs=2,
                                     name=f"ps{br}_{a}_{g}")
                    for kp in range(2):
                        for cc in range(3):
                            col = base + (g * 3 + cc) * 512
                            nc.tensor.matmul(
                                ps[:, cc * 512:(cc + 1) * 512],
                                zt3[:, 2 * kp:2 * kp + 2, lo:lo + 128],
                                zt3[:, 2 * kp:2 * kp + 2, col:col + 512],
                                start=(kp == 0), stop=(kp == 1),
                                perf_mode=DR)
                    if g == 0 or g == 2:
                        off = a * 128 if g == 0 else 1024 + a * 128
                        col_s = 24 + (br * _NA + a) + (0 if g == 0 else 8)
                        junk = pool.tile([128, 128], f32, tag="junk",
                                         bufs=2, name=f"junk{br}_{a}_{g}")
                        nc.vector.scalar_tensor_tensor(
                            out=junk[:], in0=ps[:, off:off + 128], scalar=1.0,
                            in1=ident[:], op0=Alu.mult, op1=Alu.mult,
                            accum_out=rsp[:, col_s:col_s + 1])
                    slot = (br * _NA + a) * _NG + g
                    nc.scalar.activation(ebp[g][:, ai, :], ps[:], Act.Exp,
                                         bias=0.0, scale=_A,
                                         accum_out=rsp[:, slot:slot + 1])
                    step += 1
                    while pending and pending[0][0] <= step:
                        _, pbr, pap, pg, pebt = pending.pop(0)
                        emit_colsums(pbr, pap, pg, pebt)
                pending.append((step + 3, br, apair, g, ebp[g]))
    while pending:
        _, pbr, pap, pg, pebt = pending.pop(0)
        emit_colsums(pbr, pap, pg, pebt)

    # ---- output DMA ----
    nc.sync.dma_start(rsp_out[:, :], rsp[:])
    nc.sync.dma_start(cs_out[:, 14 * 512:], cs_sb[0:1, 14 * 512:])

    ctx.close()


def build():
    import concourse.tile as tile
    from concourse import bacc, mybir

    nc = bacc.Bacc("TRN2", target_bir_lowering=False, debug=False,
                   enable_asserts=True, num_devices=_NCORES)
    ztq = nc.dram_tensor("ztq", [128, _NK * _W], mybir.dt.float8e4,
                         kind="ExternalInput").ap()
    rsp_out = nc.dram_tensor("rsp", [128, 40], mybir.dt.float32,
                             kind="ExternalOutput").ap()
    cs_out = nc.dram_tensor("cs", [1, _NCS * 512], mybir.dt.float32,
                            kind="ExternalOutput").ap()
    with tile.TileContext(nc) as tc:
        _emit(tc, ztq, rsp_out, cs_out)
    nc.compile()
    return nc


_NC_CACHE = None


def _get_nc():
    global _NC_CACHE
    if _NC_CACHE is None:
        _NC_CACHE = build()
    return _NC_CACHE


def make_in_maps(proj_1, proj_2):
    z = np.concatenate([np.asarray(proj_1, dtype=np.float32),
                        np.asarray(proj_2, dtype=np.float32)], axis=0)
    n = np.sqrt((z * z).sum(axis=1, keepdims=True))
    zq = ((z / np.maximum(n, 1e-12)) * _S).astype(ml_dtypes.float8_e4m3)
    in_maps = []
    for c in range(_NCORES):
        zr = np.roll(zq, -_RPC * c, axis=0)          # [8192, 512]
        zt = zr.T.reshape(_NK, 128, _N2)             # [k, p, col]
        ztq = np.ascontiguousarray(
            zt[:, :, 0:_W].transpose(1, 0, 2).reshape(128, _NK * _W))
        in_maps.append({"ztq": ztq})
    return in_maps


def _combine(results):
    """Host-side gather: un-rotate per-core partial sums, finish lse."""
    totals = np.zeros(_N2, dtype=np.float64)
    self_raw = np.zeros(_N2, dtype=np.float64)
    pos_raw = np.zeros(_N2, dtype=np.float64)
    p_idx = np.arange(128)
    q_idx = np.arange(512)
    for c, res in enumerate(results):
        rot = _RPC * c
        rsp = np.asarray(res["rsp"], dtype=np.float64)    # [128, 40]
        cs = np.asarray(res["cs"], dtype=np.float64)[0]   # [14336]
        for br in range(_NBR):
            for apair in range(2):
                slot0 = (br * 2 + apair) * 7
                for g in range(_NG):
                    for j in _CS_CHUNKS[g]:
                        idx = slot0 + _CS_POS[(g, j)]
                        b = br + j  # local column block
                        gr = (512 * b + q_idx + rot) % _N2
                        totals[gr] += cs[idx * 512:(idx + 1) * 512]
        for br in range(_NBR):
            for a in range(_NA):
                gr = (512 * br + 128 * a + p_idx + rot) % _N2
                s = br * _NA + a
                totals[gr] += rsp[:, s * _NG:(s + 1) * _NG].sum(axis=1)
                self_raw[gr] = rsp[:, 24 + s]
                pos_raw[gr] = rsp[:, 32 + s]
    lse = np.log(totals - np.exp(_A * self_raw))
    loss = np.mean(lse - _A * pos_raw)
    return np.float32(loss)


def kernel(proj_1, proj_2):
    from concourse import bass_utils

    nc = _get_nc()
    in_maps = make_in_maps(proj_1, proj_2)
    r = bass_utils.run_bass_kernel_spmd(nc, in_maps,
                                        core_ids=list(range(_NCORES)))
    return _combine(r.results)


# revision 19
# speedup vs baseline: 1.5647x; 1.0181x over previous
"""NT-Xent / contrastive loss on 8 Trainium2 NeuronCores.

Reference computation (B=4096, D=512, temp=0.1):
    z   = l2norm(concat(proj_1, proj_2))          # [8192, 512]
    cos = (z @ z.T) / temp                        # [8192, 8192]
    pos[r]  = cos[r, (r + 4096) % 8192]
    lse[r]  = logsumexp(cos[r, :] with cos[r, r] masked out)
    loss    = mean(lse - pos)

cos is symmetric, so each unordered pair of 512-row blocks is computed
ONCE: with the 16x16 grid of 512x512 blocks, core c covers local block
rows {0, 1} x local column blocks {br..br+8} on input *rotated* by
c*1024 rows.  The union over cores covers every unordered block pair
{B, B+j} exactly once for j=1..8 (j=8 pairs are computed from both
orientations, each contributing only its own rows' sums), and every
diagonal block once.  Row sums for the computed half come from ScalarE
accum_out; the transposed half's contributions are column sums via
fp8-e5m2 ones-matmuls (DoubleRow, two 128-row planes per instruction).
The host gathers per-core partial vectors, un-rotates, adds the row- and
column-sum halves, and finishes the (tiny) lse/loss reduction in fp64.

Device-side layout per core (all in local rotated coordinates):
  - input: z^T normalized, scaled by S=64, fp8-e4m3, GEMM-ready
    [128, 4*5120] (K-chunk k at columns [k*5120,(k+1)*5120)) -- only
    columns 0..5119 are touched (block rows 0,1 x their 9 chunks).
    Three 3D-AP DMAs (all 4 K-chunks per trigger) keep the Sync engine
    off the critical path.
  - for block-row br in {0,1}, row-subtile a in 0..3: 3 column groups
    of 1536 (9 chunks of 512: local blocks br..br+8): 6 fp8-e4m3
    DoubleRow matmuls -> [128,1536] PSUM; self diag (group 0, offset
    a*128) and positive diag (group 2, offset 1024+a*128) are pulled
    from raw PSUM with multiply-by-identity reduces (pure reads, no
    WAR: exp writes to SBUF); ScalarE Exp(scale=10/S^2) writes e5m2 to
    SBUF with accum_out row sums.
  - column sums: for chunks j=1..7 (j=0 self-block and j=8 positive
    chunk are row-sum-only), a ones-DoubleRow matmul over the two
    row-subtile planes -> [128,512] PSUM (all partitions equal), DVE
    evacuates partition 0 into a packed [1, 28*512] vector.  Each
    group's column sums are emitted two GEMM steps late so PE never
    waits on ScalarE and the DVE evacuations spread out.
Outputs: rsp [128,40] (24 row sums | 8 raw self diag | 8 raw pos
diag), cs [1,14336] (column sums).  ~80KB total DMA out.

fp8-e4m3 GEMM error keeps the loss within ~1e-4 of fp64; e5m2 on the
exp'd values only feeds column sums (relative rms error ~6%/sqrt(4096)
per row total).  The self term is subtracted on the host as
exp(A*self_raw) with A=10/S^2, matching the ScalarE exp of the same
fp32 PSUM value to table accuracy, harmless at the 2e-2 gate.
"""

import sys

import ml_dtypes
import numpy as np

if "/opt/trn_rl_repo" not in sys.path:
    sys.path.insert(0, "/opt/trn_rl_repo")

_B = 4096
_D = 512
_N2 = 2 * _B            # 8192 rows of the similarity matrix
_NCORES = 8
_RPC = _N2 // _NCORES   # 1024 rows per core
_INV_TEMP = 10.0
_S = 64.0               # fp8 quantization scale for normalized z
_A = _INV_TEMP / (_S * _S)   # exp() scale on raw fp8 GEMM accumulators

_NK = _D // 128         # 4 contraction chunks
_NBR = 2                # 512-row block-rows per core
_NA = 4                 # 128-row subtiles per block-row
_NG = 3                 # column groups per (br, a)
_GW = 1536              # columns per group (3 chunks of 512)
_W = 5120               # zT columns resident per core (blocks 0..9)

# column-sum chunks per group (skip j=0 self block and j=8 positive)
_CS_CHUNKS = {0: (1, 2), 1: (3, 4, 5), 2: (6, 7)}
_CS_POS = {}            # (g, j) -> slot within a (br, apair) 7-vector
for _g in range(_NG):
    for _j in _CS_CHUNKS[_g]:
        _CS_POS[(_g, _j)] = len(_CS_POS)
_NCS = 4 * 7            # (br, apair) combos x 7 chunks


def _emit(tc, ztq, rsp_out, cs_out):
    import concourse.bass as bass  # noqa: F401
    from concourse import mybir

    nc = tc.nc
    f32 = mybir.dt.float32
    e5 = mybir.dt.float8e5
    Alu = mybir.AluOpType
    Act = mybir.ActivationFunctionType
    DR = mybir.MatmulPerfMode.DoubleRow

    from contextlib import ExitStack
    ctx = ExitStack()
    pool = ctx.enter_context(tc.tile_pool(name="work", bufs=1))
    pers = ctx.enter_context(tc.tile_pool(name="pers", bufs=1))
    pspool = ctx.enter_context(tc.tile_pool(name="psum", bufs=1, space="PSUM"))

    # ---- constants ----
    ones = pers.tile([128, 128], f32, tag="ones")
    nc.vector.memset(ones[:], 1.0)
    ident = pers.tile([128, 128], f32, tag="ident")
    nc.gpsimd.affine_select(ident[:], ones[:], pattern=[[1, 128]],
                            compare_op=Alu.is_equal, fill=0.0,
                            base=0, channel_multiplier=-1)
    onese5 = pers.tile([128, 2, 128], e5, tag="onese5")
    nc.vector.memset(onese5[:], 1.0)

    # ---- persistent buffers ----
    zsb = pers.tile([128, _NK * _W], mybir.dt.float8e4, tag="zt")
    zt3 = zsb.rearrange("p (k c) -> p k c", k=_NK)
    rsp = pers.tile([128, 40], f32, tag="rsp")  # 24 rowsums | 8 self | 8 pos
    cs_sb = pers.tile([1, _NCS * 512], f32, tag="cs")

    # ---- input DMA: column ranges interleaved across the Sync and
    # GpSimd DGE queues (two hardware queues run in parallel); fine
    # 512-col granularity up front so the first GEMM group's operands
    # land as early as the queues allow, coarser ranges stream behind
    # compute ----
    ztq3 = ztq.rearrange("p (k c) -> p k c", k=_NK)
    ranges = [(0, 512, nc.sync), (512, 1024, nc.gpsimd),
              (1024, 1536, nc.scalar), (1536, 2560, nc.sync),
              (2560, 3584, nc.gpsimd), (3584, 4608, nc.sync),
              (4608, _W, nc.gpsimd)]
    for c0, c1, eng in ranges:
        eng.dma_start(zt3[:, :, c0:c1], ztq3[:, :, c0:c1])

    # ---- PE clock warm-up: throwaway DoubleRow matmuls on constants
    # while the first DMA is in flight (PE ramps 0.65 -> 2.4 GHz) ----
    for i in range(6):
        warm = pspool.tile([128, 512], f32, tag="cs", bufs=2,
                           name=f"warm{i}")
        nc.tensor.matmul(warm[:, 0:128], onese5[:], onese5[:],
                         start=True, stop=True, perf_mode=DR)

    # ---- main loop ----
    def emit_colsums(br, apair, g, ebt):
        slot0 = (br * 2 + apair) * 7
        for j in _CS_CHUNKS[g]:
            cchunk = j - 3 * g
            idx = slot0 + _CS_POS[(g, j)]
            cst = pspool.tile([128, 512], f32, tag="cs", bufs=2,
                              name=f"cs{br}_{apair}_{j}")
            nc.tensor.matmul(
                cst[:], onese5[:],
                ebt[:, :, cchunk * 512:(cchunk + 1) * 512],
                start=True, stop=True, perf_mode=DR)
            nc.vector.tensor_copy(
                cs_sb[0:1, idx * 512:(idx + 1) * 512], cst[0:1, :])

    pending = []        # (ready_step, br, apair, g, ebp tile)
    step = 0
    for br in range(_NBR):
        base = br * 512
        for apair in range(2):
            if br == 1 and apair == 1:
                # br0 column sums are all evacuated by now; drain them
                # early so the end-of-kernel DMA tail only carries br1
                nc.sync.dma_start(cs_out[:, 0:14 * 512],
                                  cs_sb[0:1, 0:14 * 512])
            ebp = []
            for g in range(_NG):
                ebp.append(pool.tile([128, 2, _GW], e5, tag=f"eb{g}",
                                     bufs=2, name=f"eb{br}_{apair}_{g}"))
            for g in range(_NG):
                for ai in range(2):
                    a = apair * 2 + ai
                    lo = base + a * 128
                    ps = pspool.tile([128, _GW], f32, tag="ps", bufs=2,
                                     name=f"ps{br}_{a}_{g}")
                    for kp in range(2):
                        for cc in range(3):
                            col = base + (g * 3 + cc) * 512
                            nc.tensor.matmul(
                                ps[:, cc * 512:(cc + 1) * 512],
                                zt3[:, 2 * kp:2 * kp + 2, lo:lo + 128],
                                zt3[:, 2 * kp:2 * kp + 2, col:col + 512],
                                start=(kp == 0), stop=(kp == 1),
                                perf_mode=DR)
                    if g == 0 or g == 2:
                        off = a * 128 if g == 0 else 1024 + a * 128
                        col_s = 24 + (br * _NA + a) + (0 if g == 0 else 8)
                        junk = pool.tile([128, 128], f32, tag="junk",
                                         bufs=2, name=f"junk{br}_{a}_{g}")
                        nc.vector.scalar_tensor_tensor(
                            out=junk[:], in0=ps[:, off:off + 128], scalar=1.0,
                            in1=ident[:], op0=Alu.mult, op1=Alu.mult,
                            accum_out=rsp[:, col_s:col_s + 1])
                    slot = (br * _NA + a) * _NG + g
                    nc.scalar.activation(ebp[g][:, ai, :], ps[:], Act.Exp,
                                         bias=0.0, scale=_A,
                                         accum_out=rsp[:, slot:slot + 1])
                    step += 1
                    while pending and pending[0][0] <= step:
                        _, pbr, pap, pg, pebt = pending.pop(0)
                        emit_colsums(pbr, pap, pg, pebt)
                    if step == 22:
                        # br1/apair0 column sums are evacuated by now
                        nc.sync.dma_start(cs_out[:, 14 * 512:21 * 512],
                                          cs_sb[0:1, 14 * 512:21 * 512])
                lag = 2 if (br == 1 and apair == 1) else 3
                pending.append((step + lag, br, apair, g, ebp[g]))
    while pending:
        _, pbr, pap, pg, pebt = pending.pop(0)
        emit_colsums(pbr, pap, pg, pebt)

    # ---- output DMA ----
    nc.sync.dma_start(rsp_out[:, :], rsp[:])
    nc.sync.dma_start(cs_out[:, 21 * 512:], cs_sb[0:1, 21 * 512:])

    ctx.close()


def build():
    import concourse.tile as tile
    from concourse import bacc, mybir

    nc = bacc.Bacc("TRN2", target_bir_lowering=False, debug=False,
                   enable_asserts=True, num_devices=_NCORES)
    ztq = nc.dram_tensor("ztq", [128, _NK * _W], mybir.dt.float8e4,
                         kind="ExternalInput").ap()
    rsp_out = nc.dram_tensor("rsp", [128, 40], mybir.dt.float32,
                             kind="ExternalOutput").ap()
    cs_out = nc.dram_tensor("cs", [1, _NCS * 512], mybir.dt.float32,
                            kind="ExternalOutput").ap()
    with tile.TileContext(nc) as tc:
        _emit(tc, ztq, rsp_out, cs_out)
    nc.compile()
    return nc


_NC_CACHE = None


def _get_nc():
    global _NC_CACHE
    if _NC_CACHE is None:
        _NC_CACHE = build()
    return _NC_CACHE


def make_in_maps(proj_1, proj_2):
    z = np.concatenate([np.asarray(proj_1, dtype=np.float32),
                        np.asarray(proj_2, dtype=np.float32)], axis=0)
    n = np.sqrt((z * z).sum(axis=1, keepdims=True))
    zq = ((z / np.maximum(n, 1e-12)) * _S).astype(ml_dtypes.float8_e4m3)
    in_maps = []
    for c in range(_NCORES):
        zr = np.roll(zq, -_RPC * c, axis=0)          # [8192, 512]
        zt = zr.T.reshape(_NK, 128, _N2)             # [k, p, col]
        ztq = np.ascontiguousarray(
            zt[:, :, 0:_W].transpose(1, 0, 2).reshape(128, _NK * _W))
        in_maps.append({"ztq": ztq})
    return in_maps


def _combine(results):
    """Host-side gather: un-rotate per-core partial sums, finish lse."""
    totals = np.zeros(_N2, dtype=np.float64)
    self_raw = np.zeros(_N2, dtype=np.float64)
    pos_raw = np.zeros(_N2, dtype=np.float64)
    p_idx = np.arange(128)
    q_idx = np.arange(512)
    for c, res in enumerate(results):
        rot = _RPC * c
        rsp = np.asarray(res["rsp"], dtype=np.float64)    # [128, 40]
        cs = np.asarray(res["cs"], dtype=np.float64)[0]   # [14336]
        for br in range(_NBR):
            for apair in range(2):
                slot0 = (br * 2 + apair) * 7
                for g in range(_NG):
                    for j in _CS_CHUNKS[g]:
                        idx = slot0 + _CS_POS[(g, j)]
                        b = br + j  # local column block
                        gr = (512 * b + q_idx + rot) % _N2
                        totals[gr] += cs[idx * 512:(idx + 1) * 512]
        for br in range(_NBR):
            for a in range(_NA):
                gr = (512 * br + 128 * a + p_idx + rot) % _N2
                s = br * _NA + a
                totals[gr] += rsp[:, s * _NG:(s + 1) * _NG].sum(axis=1)
                self_raw[gr] = rsp[:, 24 + s]
                pos_raw[gr] = rsp[:, 32 + s]
    lse = np.log(totals - np.exp(_A * self_raw))
    loss = np.mean(lse - _A * pos_raw)
    return np.float32(loss)


def kernel(proj_1, proj_2):
    from concourse import bass_utils

    nc = _get_nc()
    in_maps = make_in_maps(proj_1, proj_2)
    r = bass_utils.run_bass_kernel_spmd(nc, in_maps,
                                        core_ids=list(range(_NCORES)))
    return _combine(r.results)
